# revision 1
# baseline (speedup 1.0000x reference)
"""Multi-head causal attention (B=2, S=2048, D=1024, H=16) on 8 trn2 cores.

Sharding: core c handles batch b = c // 4 and head group g = c % 4 (4 heads,
256 feature columns). Each core computes its heads' attention context and a
partial output projection (ctx_g @ Wo[rows_g]); the host sums the 4 partials
per batch and adds bo.

Per-core kernel layout choices (all matmuls in fp32r):
- x is pre-transposed on the host to xT [D, S] so the contraction dim (d) of
  the QKV projections sits on SBUF partitions with no on-device transposes.
- Q^T, K^T [256, S] are produced head-major so scores can be computed in
  transposed layout S^T[sk, sq] = K @ Q^T; then P^T = exp(S^T) is directly the
  moving operand of ctx^T = (V|1)^T.T @ P^T, so flash-style PV needs no
  transpose either.
- Softmax: scores/8 are small (|s|<~3), so exp without max subtraction is
  safe; the denominator comes from a ones column folded into the V stationary
  operand; normalization multiplies ctx^T by a DMA-broadcast reciprocal row.
- Causal mask: gpsimd affine_select zeroes p^T entries with sk > sq on the 4
  diagonal tiles of each (head, sq-tile); fully-masked tiles are skipped.
"""

import os
import sys
import types
from contextlib import ExitStack

import numpy as np

import concourse.bacc as bacc
import concourse.bass as bass
import concourse.mybir as mybir
import concourse.tile as tile
from concourse.bass_utils import run_bass_kernel_spmd


def _install_ntff_hook():
    """The agent image's antenv lacks axon_hooks, so trn_boot's NTFF hook
    install degrades silently. Recreate the module + hook so trace=True works."""
    if "antenv.axon_hooks" in sys.modules:
        return
    try:
        mod = types.ModuleType("antenv.axon_hooks")
        holder = [None]
        mod.set_axon_ntff_profile_hook = lambda h: holder.__setitem__(0, h)
        mod.get_axon_ntff_profile_hook = lambda: holder[0]
        from trn_agent_boot.trn_boot import _ntff_profile_via_ctypes

        hook = _ntff_profile_via_ctypes("/opt/axon/libaxon_pjrt.so")
        if hook is None:
            return
        mod.set_axon_ntff_profile_hook(hook)
        sys.modules["antenv.axon_hooks"] = mod
    except Exception:
        pass

B, S, D, H, HD = 2, 2048, 1024, 16, 64
NCORES = 8
GROUPS = 4          # head groups (cores) per batch
HC = H // GROUPS    # heads per core
DG = HC * HD        # feature columns per core (256)
P = 128
KSUB = D // P       # 8 contraction subtiles for the projections
SQT = 512           # sq tile width (free dim of scores/ctx matmuls)
NSQ = S // SQT      # 4
NST = S // P        # 16 s subtiles of 128
F32 = mybir.dt.float32
F32R = mybir.dt.float32r

_CACHE = {}


def _mha_tile_kernel(tc, xT, wq, wk, wv, wo, out):
    nc = tc.nc
    scale = 1.0 / np.sqrt(np.float32(HD))

    with ExitStack() as ctx:
        consts = ctx.enter_context(tc.tile_pool(name="consts", bufs=1))
        dramp = ctx.enter_context(tc.tile_pool(name="dramp", bufs=3, space="DRAM"))
        # PSUM: two 2-bank [128,1024] working tiles + four 1-bank ctx accumulators
        sps = ctx.enter_context(tc.tile_pool(name="sps", bufs=2, space="PSUM"))
        cps = ctx.enter_context(tc.tile_pool(name="cps", bufs=4, space="PSUM"))
        # x slices + rotating QKV weights; released after the projections so
        # the attention-phase pools reuse the space
        xw = tc.alloc_tile_pool(name="xw", bufs=1)

        # --- persistent SBUF tensors ---
        wo_sb = consts.tile([P, DG // P, D], F32R)
        nc.sync.dma_start(out=wo_sb, in_=wo)
        wq_sb = xw.tile([P, KSUB, DG], F32R, tag="w", bufs=3, name="wq_sb")
        wk_sb = xw.tile([P, KSUB, DG], F32R, tag="w", bufs=3, name="wk_sb")
        wv_sb = xw.tile([P, KSUB, DG], F32R, tag="w", bufs=3, name="wv_sb")
        nc.sync.dma_start(out=wq_sb, in_=wq)
        nc.sync.dma_start(out=wk_sb, in_=wk)
        nc.sync.dma_start(out=wv_sb, in_=wv)

        qt_sb = consts.tile([P, DG // P, S], F32R)   # Q^T: head h at [64*(h%2):, h//2, :]
        # K^T zero-padded per head: head h's 64 rows live at [64*(h%2):, h, :],
        # the other 64 rows are 0 so score matmuls contract over K=128 (keeps
        # the PE's HAM activity monitor engaged at full clock).
        kt_sb = consts.tile([P, HC, S], F32R)
        # V with the ones column baked in, per s-subtile and head:
        #   even h: [V(64) | 1 | 0(63)]  -> ctx rows 0-63, denom row 64
        #   odd  h: [1 | 0(63) | V(64)]  -> denom row 0, ctx rows 64-127
        v_sb = consts.tile([P, NST, HC, P], F32R)
        ctxt_sb = consts.tile([P, DG // P, S], F32R)  # normalized ctx^T, same layout as qt

        # memset can't write fp32r; broadcast-copy from small f32 scratch instead
        zsc = consts.tile([P, P], F32, tag="zsc", bufs=1)
        nc.vector.memset(zsc, 0.0)
        osc = consts.tile([P, 1], F32, tag="osc", bufs=1)
        nc.vector.memset(osc, 1.0)
        nc.vector.tensor_copy(
            out=v_sb, in_=zsc[:, None, None, :].to_broadcast((P, NST, HC, P))
        )
        nc.vector.tensor_copy(
            out=kt_sb.rearrange("p h (a b) -> p h a b", b=P),
            in_=zsc[:, None, None, :].to_broadcast((P, HC, S // P, P)),
        )
        for h in range(HC):
            ones_col = 64 if h % 2 == 0 else 0
            nc.vector.tensor_copy(
                out=v_sb[:, :, h, ones_col : ones_col + 1],
                in_=osc[:, None, :].to_broadcast((P, NST, 1)),
            )

        # --- phase 1+2: stream xT by sq-slice; QT/KT/V interleaved per slice
        # so attention tiles unblock as soon as slice 0 is projected.
        for n in range(NSQ):
            nsl = slice(n * SQT, (n + 1) * SQT)
            xn = xw.tile([P, KSUB, SQT], F32R, tag="xT", bufs=3, name=f"xn_{n}")
            for k in range(KSUB):
                nc.sync.dma_start(
                    out=xn[:, k, :], in_=xT[k * P : (k + 1) * P, n * SQT : (n + 1) * SQT]
                )
            ps = sps.tile([P, 2 * SQT], F32, tag="s", name=f"qps_{n}")
            for m in range(DG // P):
                for k in range(KSUB):
                    nc.tensor.matmul(
                        ps[:, m * SQT : (m + 1) * SQT],
                        lhsT=wq_sb[:, k, m * P : (m + 1) * P],
                        rhs=xn[:, k, :],
                        start=(k == 0),
                        stop=(k == KSUB - 1),
                    )
            nc.vector.tensor_copy(
                out=qt_sb[:, :, nsl],
                in_=ps.rearrange("p (m f) -> p m f", f=SQT),
            )
            ps = sps.tile([P, 2 * SQT], F32, tag="s", name=f"kps_{n}")
            for m in range(DG // P):
                for k in range(KSUB):
                    nc.tensor.matmul(
                        ps[:, m * SQT : (m + 1) * SQT],
                        lhsT=wk_sb[:, k, m * P : (m + 1) * P],
                        rhs=xn[:, k, :],
                        start=(k == 0),
                        stop=(k == KSUB - 1),
                    )
            psv = ps.rearrange("p (m f) -> p m f", f=SQT)
            nc.vector.tensor_copy(out=kt_sb[0:64, 0::2, nsl], in_=psv[0:64, :, :])
            nc.vector.tensor_copy(out=kt_sb[64:P, 1::2, nsl], in_=psv[64:P, :, :])
            ps = sps.tile([P, 2 * SQT], F32, tag="s", name=f"vps_{n}")
            for sst in range(SQT // P):
                for k in range(KSUB):
                    nc.tensor.matmul(
                        ps[:, sst * DG : (sst + 1) * DG],
                        lhsT=xn[:, k, sst * P : (sst + 1) * P],
                        rhs=wv_sb[:, k, :],
                        start=(k == 0),
                        stop=(k == KSUB - 1),
                    )
            st0 = n * (SQT // P)
            # psum view: [128, st(4), h(4), 64]; even heads -> cols 0:64,
            # odd heads -> cols 64:128 of the padded V layout
            psv = ps.rearrange("p (t h d) -> p t h d", h=HC, d=HD)
            nc.vector.tensor_copy(
                out=v_sb[:, st0 : st0 + 4, 0:HC:2, 0:HD], in_=psv[:, :, 0:HC:2, :]
            )
            nc.vector.tensor_copy(
                out=v_sb[:, st0 : st0 + 4, 1:HC:2, HD:P], in_=psv[:, :, 1:HC:2, :]
            )

        xw.release()
        ptp = ctx.enter_context(tc.tile_pool(name="ptp", bufs=6))
        smalls = ctx.enter_context(tc.tile_pool(name="smalls", bufs=3))
        outp = ctx.enter_context(tc.tile_pool(name="outp", bufs=3))

        # --- phase 3: attention, sk-tile-major; the up-to-4 sq-tiles per
        # sk-tile are independent chains that keep the PE dense. Scores/exp/PV
        # windowed to valid columns [w0:512]; sq-tile pairs share one 2-bank
        # psum tile so exp runs as one wide ACTIVATE.
        for h in range(HC):
            hm = h // 2
            hp = 64 * (h % 2)
            ctx_rows = 0 if h % 2 == 0 else 64
            denom_row = 64 if h % 2 == 0 else 0
            cpsums = [
                cps.tile([P, SQT], F32, tag="ctx", name=f"ctx_{h}_{i}")
                for i in range(NSQ)
            ]
            for ski in range(NST):
                sqts = list(range(ski // 4, NSQ))
                pts = []
                for pair0 in range(0, len(sqts), 2):
                    grp = sqts[pair0 : pair0 + 2]
                    spsum = sps.tile(
                        [P, 2 * SQT], F32, tag="s", name=f"s_{h}_{ski}_{pair0}"
                    )
                    pt = ptp.tile(
                        [P, 2 * SQT], F32R, tag="pt", name=f"pt_{h}_{ski}_{pair0}"
                    )
                    w0g = None
                    for jj, sqt in enumerate(grp):
                        sq0 = sqt * SQT
                        diag = ski >= 4 * sqt
                        w0 = (128 * ski - sq0) if diag else 0
                        if w0g is None:
                            w0g = jj * SQT + w0
                        base = jj * SQT
                        nc.tensor.matmul(
                            spsum[:, base + w0 : base + SQT],
                            lhsT=kt_sb[:, h, ski * P : (ski + 1) * P],
                            rhs=qt_sb[:, hm, sq0 + w0 : sq0 + SQT],
                            start=True,
                            stop=True,
                        )
                        pts.append((sqt, w0, pt, base, diag))
                    wend = (len(grp) - 1) * SQT + SQT
                    nc.scalar.activation(
                        out=pt[:, w0g:wend], in_=spsum[:, w0g:wend],
                        func=mybir.ActivationFunctionType.Exp,
                        bias=0.0, scale=float(scale),
                    )
                for sqt, w0, pt, base, diag in pts:
                    if diag:  # zero entries with sk > sq in the triangular block
                        nc.gpsimd.affine_select(
                            out=pt[:, base + w0 : base + w0 + P],
                            in_=pt[:, base + w0 : base + w0 + P],
                            pattern=[[1, P]],
                            compare_op=mybir.AluOpType.is_ge,
                            fill=0.0,
                            base=0,
                            channel_multiplier=-1,
                        )
                for sqt, w0, pt, base, diag in pts:
                    nc.tensor.matmul(
                        cpsums[sqt][:, w0:],
                        lhsT=v_sb[:, ski, h, :],
                        rhs=pt[:, base + w0 : base + SQT],
                        start=(ski == 0),
                        stop=(ski == 4 * sqt + 3),
                    )
                    if ski == 4 * sqt + 3:
                        # normalize eagerly once this sq-tile's chain stops:
                        # ctx rows *= 1/denom (broadcast across partitions).
                        sq0 = sqt * SQT
                        cpsum = cpsums[sqt]
                        rec_t = smalls.tile([P, SQT], F32, tag="recip")
                        nc.vector.tensor_copy(
                            out=rec_t[denom_row : denom_row + 1, :],
                            in_=cpsum[denom_row : denom_row + 1, :],
                        )
                        # partition-scatter so reciprocal uses all DVE lanes
                        spread = smalls.tile([P, SQT // P], F32, tag="spread")
                        nc.sync.dma_start(
                            out=spread, in_=rec_t[denom_row : denom_row + 1, :]
                        )
                        nc.vector.reciprocal(out=spread, in_=spread)
                        rec_d2 = dramp.tile([1, SQT], F32, tag="rec_d2")
                        nc.sync.dma_start(
                            out=rec_d2.rearrange("a (p f) -> (a p) f", p=P),
                            in_=spread,
                        )
                        bcast = smalls.tile([P, SQT], F32, tag="bcast")
                        rec_b = bass.AP(
                            tensor=rec_d2.tensor,
                            offset=rec_d2.offset,
                            ap=[[0, 64]] + [list(p) for p in rec_d2.ap[1:]],
                        )
                        nc.sync.dma_start(
                            out=bcast[ctx_rows : ctx_rows + 64, :], in_=rec_b
                        )
                        nc.vector.tensor_tensor(
                            ctxt_sb[hp : hp + 64, hm, sq0 : sq0 + SQT],
                            cpsum[ctx_rows : ctx_rows + 64, :],
                            bcast[ctx_rows : ctx_rows + 64, :],
                            mybir.AluOpType.mult,
                        )

        # --- phase 4: partial output projection out = ctx @ Wo_slice ---
        for st in range(NST):
            ot = outp.tile([P, D], F32, tag="out")
            for nn in range(D // SQT):
                ps = cps.tile([P, SQT], F32, tag="ctx", name=f"ops_{st}_{nn}")
                for k in range(DG // P):
                    nc.tensor.matmul(
                        ps,
                        lhsT=ctxt_sb[:, k, st * P : (st + 1) * P],
                        rhs=wo_sb[:, k, nn * SQT : (nn + 1) * SQT],
                        start=(k == 0),
                        stop=(k == DG // P - 1),
                    )
                nc.vector.tensor_copy(out=ot[:, nn * SQT : (nn + 1) * SQT], in_=ps)
            nc.scalar.dma_start(out=out[st * P : (st + 1) * P, :], in_=ot)


def build_nc():
    if "nc" in _CACHE:
        return _CACHE["nc"]
    nc = bacc.Bacc("TRN2", target_bir_lowering=False, debug=False, num_devices=NCORES)
    xT = nc.dram_tensor("xT", (D, S), F32R, kind="ExternalInput").ap()
    wq = nc.dram_tensor("wq", (P, KSUB, DG), F32R, kind="ExternalInput").ap()
    wk = nc.dram_tensor("wk", (P, KSUB, DG), F32R, kind="ExternalInput").ap()
    wv = nc.dram_tensor("wv", (P, KSUB, DG), F32R, kind="ExternalInput").ap()
    wo = nc.dram_tensor("wo", (P, DG // P, D), F32R, kind="ExternalInput").ap()
    out = nc.dram_tensor("out", (S, D), F32, kind="ExternalOutput").ap()
    with tile.TileContext(nc) as tc:
        _mha_tile_kernel(tc, xT, wq, wk, wv, wo, out)
    nc.compile()
    _CACHE["nc"] = nc
    return nc


def make_in_maps(x, Wq, Wk, Wv, Wo):
    x = np.asarray(x, np.float32)
    in_maps = []
    for c in range(NCORES):
        b, g = c // GROUPS, c % GROUPS
        cols = slice(g * DG, (g + 1) * DG)

        def wslice(W):
            # [D, DG] -> [128, KSUB, DG] with [p, k, m] = W[k*128+p, m]
            return np.ascontiguousarray(
                np.asarray(W, np.float32)[:, cols].reshape(KSUB, P, DG).transpose(1, 0, 2)
            )

        wo_c = np.ascontiguousarray(
            np.asarray(Wo, np.float32)[cols, :].reshape(DG // P, P, D).transpose(1, 0, 2)
        )
        in_maps.append(
            {
                "xT": np.ascontiguousarray(x[b].T),
                "wq": wslice(Wq),
                "wk": wslice(Wk),
                "wv": wslice(Wv),
                "wo": wo_c,
            }
        )
    return in_maps


def kernel(x, Wq, Wk, Wv, Wo, bo):
    nc = build_nc()
    in_maps = make_in_maps(x, Wq, Wk, Wv, Wo)
    trace = bool(int(os.environ.get("MHA_TRACE", "0")))
    if trace:
        _install_ntff_hook()
    res = run_bass_kernel_spmd(
        nc, in_maps, core_ids=list(range(NCORES)), trace=trace,
        trace_cores=list(range(NCORES)) if trace else None,
    )
    _CACHE["last_results"] = res
    bo = np.asarray(bo, np.float32)
    out = np.zeros((B, S, D), np.float32)
    for c in range(NCORES):
        out[c // GROUPS] += res.results[c]["out"]
    out += bo[None, None, :]
    return out



# revision 2
# speedup vs baseline: 1.0144x; 1.0144x over previous
"""Multi-head causal attention (B=2, S=2048, D=1024, H=16) on 8 trn2 cores.

Sharding: core c handles batch b = c // 4 and head group g = c % 4 (4 heads,
256 feature columns). Each core computes its heads' attention context and a
partial output projection (ctx_g @ Wo[rows_g]); the host sums the 4 partials
per batch and adds bo.

v2 layout (all matmul operands bf16, fp32 psum accumulation):
- x is pre-transposed+cast on the host to xT [D, S] bf16; weights bf16.
- DMAs issued in demand order (wq k0 first, wo last) so the first projection
  matmul starts ~2us in instead of ~24us.
- Attention is sq-tile-major: after each sq-tile's 4 heads finish, its output
  projection is interleaved with the next projection slice, so output DMA
  overlaps compute instead of draining at the end.
- Within a sq-tile, ski pairs share one 2-bank psum + one wide exp ACTIVATE;
  score matmuls of pair i+1 are issued before PV of pair i so the PE never
  stalls on the ACT latency.
- Causal diag masking via DVE multiply with a precomputed triangular bf16
  mask (gpsimd affine_select only used once to build the mask).
- Softmax denominator via ones column folded into the V stationary; the
  reciprocal row is broadcast across partitions with a DRAM round-trip DMA.
"""

import os
import sys
import types
from contextlib import ExitStack

import numpy as np
import ml_dtypes

import concourse.bacc as bacc
import concourse.bass as bass
import concourse.mybir as mybir
import concourse.tile as tile
from concourse.bass_utils import run_bass_kernel_spmd


def _install_ntff_hook():
    """The agent image's antenv lacks axon_hooks, so trn_boot's NTFF hook
    install degrades silently. Recreate the module + hook so trace=True works."""
    if "antenv.axon_hooks" in sys.modules:
        return
    try:
        mod = types.ModuleType("antenv.axon_hooks")
        holder = [None]
        mod.set_axon_ntff_profile_hook = lambda h: holder.__setitem__(0, h)
        mod.get_axon_ntff_profile_hook = lambda: holder[0]
        from trn_agent_boot.trn_boot import _ntff_profile_via_ctypes

        hook = _ntff_profile_via_ctypes("/opt/axon/libaxon_pjrt.so")
        if hook is None:
            return
        mod.set_axon_ntff_profile_hook(hook)
        sys.modules["antenv.axon_hooks"] = mod
    except Exception:
        pass

B, S, D, H, HD = 2, 2048, 1024, 16, 64
NCORES = 8
GROUPS = 4          # head groups (cores) per batch
HC = H // GROUPS    # heads per core
DG = HC * HD        # feature columns per core (256)
P = 128
KSUB = D // P       # 8 contraction subtiles for the projections
SQT = 512           # sq tile width (free dim of scores/ctx matmuls)
NSQ = S // SQT      # 4
NST = S // P        # 16 s subtiles of 128
F32 = mybir.dt.float32
BF16 = mybir.dt.bfloat16
BFNP = ml_dtypes.bfloat16

_CACHE = {}


def _mha_tile_kernel(tc, xT, wq, wk, wv, wo, out):
    nc = tc.nc
    scale = 1.0 / np.sqrt(np.float32(HD))

    with ExitStack() as ctx:
        consts = ctx.enter_context(tc.tile_pool(name="consts", bufs=1))
        dramp = ctx.enter_context(tc.tile_pool(name="dramp", bufs=3, space="DRAM"))
        # PSUM: 2x 2-bank score tiles + 2 ctx accumulators + 2 out-proj tiles
        sps = ctx.enter_context(tc.tile_pool(name="sps", bufs=2, space="PSUM"))
        cps = ctx.enter_context(tc.tile_pool(name="cps", bufs=2, space="PSUM"))
        ops = ctx.enter_context(tc.tile_pool(name="ops", bufs=2, space="PSUM"))
        xnp = ctx.enter_context(tc.tile_pool(name="xnp", bufs=3))
        ptp = ctx.enter_context(tc.tile_pool(name="ptp", bufs=6))
        smalls = ctx.enter_context(tc.tile_pool(name="smalls", bufs=3))
        outp = ctx.enter_context(tc.tile_pool(name="outp", bufs=3))

        # --- persistent SBUF tensors ---
        wq_sb = consts.tile([P, KSUB, DG], BF16)
        wk_sb = consts.tile([P, KSUB, DG], BF16)
        wv_sb = consts.tile([P, KSUB, DG], BF16)
        wo_sb = consts.tile([P, DG // P, D], BF16)
        qt_sb = consts.tile([P, DG // P, S], BF16)   # Q^T: head h at [64*(h%2):, h//2, :]
        # K^T zero-padded per head: head h's 64 rows at [64*(h%2):, h, :], the
        # other 64 rows 0 so score matmuls contract over K=128 (keeps the PE's
        # HAM activity monitor at full clock).
        kt_sb = consts.tile([P, HC, S], BF16)
        # V with the ones column baked in, per s-subtile and head:
        #   even h: [V(64) | 1 | 0(63)]  -> ctx rows 0-63, denom row 64
        #   odd  h: [1 | 0(63) | V(64)]  -> denom row 0, ctx rows 64-127
        v_sb = consts.tile([P, NST, HC, P], BF16)
        ctxt_sb = consts.tile([P, DG // P, S], BF16)  # normalized ctx^T, qt layout

        # demand-ordered input DMAs: wq k0 chunk, then x slice 0, then the rest
        nc.sync.dma_start(out=wq_sb[:, 0:1, :], in_=wq[:, 0:1, :])
        nc.sync.dma_start(out=wq_sb[:, 1:KSUB, :], in_=wq[:, 1:KSUB, :])

        def load_x(n):
            xn = xnp.tile([P, KSUB, SQT], BF16, tag="xT", name=f"xn_{n}")
            for k in range(KSUB):
                nc.sync.dma_start(
                    out=xn[:, k, :], in_=xT[k * P : (k + 1) * P, n * SQT : (n + 1) * SQT]
                )
            return xn

        xn0 = load_x(0)
        nc.sync.dma_start(out=wk_sb[:, 0:1, :], in_=wk[:, 0:1, :])
        nc.sync.dma_start(out=wk_sb[:, 1:KSUB, :], in_=wk[:, 1:KSUB, :])
        nc.sync.dma_start(out=wv_sb[:, 0:1, :], in_=wv[:, 0:1, :])
        nc.sync.dma_start(out=wv_sb[:, 1:KSUB, :], in_=wv[:, 1:KSUB, :])

        # --- small constants ---
        zrow = consts.tile([P, SQT], F32, tag="zrow", bufs=1)
        nc.vector.memset(zrow, 0.0)
        osc = consts.tile([P, 1], F32, tag="osc", bufs=1)
        nc.vector.memset(osc, 1.0)
        mskf = consts.tile([P, P], F32, tag="mskf", bufs=1)
        nc.vector.memset(mskf, 1.0)
        # keep entries with col >= row (sk <= sq), zero the rest
        nc.gpsimd.affine_select(
            out=mskf, in_=mskf, pattern=[[1, P]],
            compare_op=mybir.AluOpType.is_ge, fill=0.0,
            base=0, channel_multiplier=-1,
        )
        msk = consts.tile([P, P], BF16, tag="msk", bufs=1)
        nc.vector.tensor_copy(out=msk, in_=mskf)

        def proj_slice(n, xn):
            nsl = slice(n * SQT, (n + 1) * SQT)
            ps = sps.tile([P, 2 * SQT], F32, tag="s", name=f"qps_{n}")
            for m in range(DG // P):
                for k in range(KSUB):
                    nc.tensor.matmul(
                        ps[:, m * SQT : (m + 1) * SQT],
                        lhsT=wq_sb[:, k, m * P : (m + 1) * P],
                        rhs=xn[:, k, :],
                        start=(k == 0), stop=(k == KSUB - 1),
                    )
            nc.vector.tensor_copy(
                out=qt_sb[:, :, nsl], in_=ps.rearrange("p (m f) -> p m f", f=SQT)
            )
            ps = sps.tile([P, 2 * SQT], F32, tag="s", name=f"kps_{n}")
            for m in range(DG // P):
                for k in range(KSUB):
                    nc.tensor.matmul(
                        ps[:, m * SQT : (m + 1) * SQT],
                        lhsT=wk_sb[:, k, m * P : (m + 1) * P],
                        rhs=xn[:, k, :],
                        start=(k == 0), stop=(k == KSUB - 1),
                    )
            psv = ps.rearrange("p (m f) -> p m f", f=SQT)
            nc.vector.tensor_copy(
                out=kt_sb[64:P, 0::2, nsl],
                in_=zrow[64:P, None, :].to_broadcast((64, 2, SQT)),
            )
            nc.vector.tensor_copy(
                out=kt_sb[0:64, 1::2, nsl],
                in_=zrow[0:64, None, :].to_broadcast((64, 2, SQT)),
            )
            nc.vector.tensor_copy(out=kt_sb[0:64, 0::2, nsl], in_=psv[0:64, :, :])
            nc.vector.tensor_copy(out=kt_sb[64:P, 1::2, nsl], in_=psv[64:P, :, :])
            ps = sps.tile([P, 2 * SQT], F32, tag="s", name=f"vps_{n}")
            for sst in range(SQT // P):
                for k in range(KSUB):
                    nc.tensor.matmul(
                        ps[:, sst * DG : (sst + 1) * DG],
                        lhsT=xn[:, k, sst * P : (sst + 1) * P],
                        rhs=wv_sb[:, k, :],
                        start=(k == 0), stop=(k == KSUB - 1),
                    )
            st0 = n * (SQT // P)
            psv = ps.rearrange("p (t h d) -> p t h d", h=HC, d=HD)
            # pad zeros + ones column for this slice's st range, then real V
            nc.vector.tensor_copy(
                out=v_sb[:, st0 : st0 + 4, 0:HC:2, HD + 1 : P],
                in_=zrow[:, None, None, 0 : P - HD - 1].to_broadcast((P, 4, 2, P - HD - 1)),
            )
            nc.vector.tensor_copy(
                out=v_sb[:, st0 : st0 + 4, 1:HC:2, 1 : HD],
                in_=zrow[:, None, None, 0 : HD - 1].to_broadcast((P, 4, 2, HD - 1)),
            )
            for h in range(HC):
                oc = HD if h % 2 == 0 else 0
                nc.vector.tensor_copy(
                    out=v_sb[:, st0 : st0 + 4, h, oc : oc + 1],
                    in_=osc[:, None, :].to_broadcast((P, 4, 1)),
                )
            nc.vector.tensor_copy(
                out=v_sb[:, st0 : st0 + 4, 0:HC:2, 0:HD], in_=psv[:, :, 0:HC:2, :]
            )
            nc.vector.tensor_copy(
                out=v_sb[:, st0 : st0 + 4, 1:HC:2, HD:P], in_=psv[:, :, 1:HC:2, :]
            )

        def attention(sqt):
            """sq-tile sqt: all 4 heads, ski pairs software-pipelined so the
            PE runs scores of pair i+1 while the ACT exps pair i."""
            sq0 = sqt * SQT
            nsk = 4 * sqt + 4
            pending = None  # (infos, pt, cpsum, nsk) awaiting PV emission

            def emit_pv(p):
                infos, pt, cpsum = p
                for ski, w0, base in infos:
                    nc.tensor.matmul(
                        cpsum[:, w0:],
                        lhsT=v_sb[:, ski, h_of[id(cpsum)], :],
                        rhs=pt[:, base + w0 : base + SQT],
                        start=(ski == 0), stop=(ski == nsk - 1),
                    )

            h_of = {}
            norms = []
            for h in range(HC):
                hm = h // 2
                hp = 64 * (h % 2)
                ctx_rows = 0 if h % 2 == 0 else 64
                denom_row = 64 if h % 2 == 0 else 0
                cpsum = cps.tile([P, SQT], F32, tag="ctx", name=f"ctx_{sqt}_{h}")
                h_of[id(cpsum)] = h
                for sk0 in range(0, nsk, 2):
                    spsum = sps.tile([P, 2 * SQT], F32, tag="s", name=f"s_{sqt}_{h}_{sk0}")
                    pt = ptp.tile([P, 2 * SQT], BF16, tag="pt", name=f"pt_{sqt}_{h}_{sk0}")
                    infos = []
                    for jj in range(2):
                        ski = sk0 + jj
                        diag = ski >= 4 * sqt
                        w0 = (128 * ski - sq0) if diag else 0
                        base = jj * SQT
                        nc.tensor.matmul(
                            spsum[:, base + w0 : base + SQT],
                            lhsT=kt_sb[:, h, ski * P : (ski + 1) * P],
                            rhs=qt_sb[:, hm, sq0 + w0 : sq0 + SQT],
                            start=True, stop=True,
                        )
                        infos.append((ski, w0, base, diag))
                    w0g = infos[0][1]
                    nc.scalar.activation(
                        out=pt[:, w0g : 2 * SQT], in_=spsum[:, w0g : 2 * SQT],
                        func=mybir.ActivationFunctionType.Exp,
                        bias=0.0, scale=float(scale),
                    )
                    for ski, w0, base, diag in infos:
                        if diag:
                            nc.vector.tensor_tensor(
                                pt[:, base + w0 : base + w0 + P],
                                pt[:, base + w0 : base + w0 + P],
                                msk, mybir.AluOpType.mult,
                            )
                    if pending is not None:
                        emit_pv(pending)
                    pending = ([(ski, w0, base) for ski, w0, base, _ in infos], pt, cpsum)
                # norm bookkeeping emitted after this head's last PV (below)
                norms.append((cpsum, h, hm, hp, ctx_rows, denom_row))
                if h == HC - 1 and pending is not None:
                    emit_pv(pending)
                    pending = None
                # emit the norm for the PREVIOUS head now (its last PV was just
                # emitted inside this head's first pair), and for this head if last
                while norms and (len(norms) > 1 or h == HC - 1):
                    cp, nh, nhm, nhp, ncr, ndr = norms.pop(0)
                    rt = smalls.tile([1, SQT], F32, tag="rt", name=f"rt_{sqt}_{nh}")
                    nc.vector.tensor_copy(out=rt, in_=cp[ndr : ndr + 1, :])
                    nc.vector.reciprocal(out=rt, in_=rt)
                    rec_d = dramp.tile([1, SQT], F32, tag="rec", name=f"rec_{sqt}_{nh}")
                    nc.sync.dma_start(out=rec_d, in_=rt)
                    bcast = smalls.tile([P, SQT], F32, tag="bcast", name=f"bc_{sqt}_{nh}")
                    rec_b = bass.AP(
                        tensor=rec_d.tensor, offset=rec_d.offset,
                        ap=[[0, 64]] + [list(p) for p in rec_d.ap[1:]],
                    )
                    nc.sync.dma_start(out=bcast[ncr : ncr + 64, :], in_=rec_b)
                    nc.vector.tensor_tensor(
                        ctxt_sb[nhp : nhp + 64, nhm, sq0 : sq0 + SQT],
                        cp[ncr : ncr + 64, :],
                        bcast[ncr : ncr + 64, :],
                        mybir.AluOpType.mult,
                    )

        def outproj(sqt):
            for st in range(4 * sqt, 4 * sqt + 4):
                ot = outp.tile([P, D], BF16, tag="out", name=f"ot_{st}")
                for nn in range(D // SQT):
                    ps = ops.tile([P, SQT], F32, tag="op", name=f"op_{st}_{nn}")
                    for k in range(DG // P):
                        nc.tensor.matmul(
                            ps,
                            lhsT=ctxt_sb[:, k, st * P : (st + 1) * P],
                            rhs=wo_sb[:, k, nn * SQT : (nn + 1) * SQT],
                            start=(k == 0), stop=(k == DG // P - 1),
                        )
                    nc.vector.tensor_copy(out=ot[:, nn * SQT : (nn + 1) * SQT], in_=ps)
                nc.scalar.dma_start(out=out[st * P : (st + 1) * P, :], in_=ot)

        proj_slice(0, xn0)
        xn1 = load_x(1)
        attention(0)
        nc.sync.dma_start(out=wo_sb, in_=wo)
        proj_slice(1, xn1)
        xn2 = load_x(2)
        outproj(0)
        attention(1)
        proj_slice(2, xn2)
        xn3 = load_x(3)
        outproj(1)
        attention(2)
        proj_slice(3, xn3)
        outproj(2)
        attention(3)
        outproj(3)


def build_nc():
    if "nc" in _CACHE:
        return _CACHE["nc"]
    nc = bacc.Bacc("TRN2", target_bir_lowering=False, debug=False, num_devices=NCORES)
    xT = nc.dram_tensor("xT", (D, S), BF16, kind="ExternalInput").ap()
    wq = nc.dram_tensor("wq", (P, KSUB, DG), BF16, kind="ExternalInput").ap()
    wk = nc.dram_tensor("wk", (P, KSUB, DG), BF16, kind="ExternalInput").ap()
    wv = nc.dram_tensor("wv", (P, KSUB, DG), BF16, kind="ExternalInput").ap()
    wo = nc.dram_tensor("wo", (P, DG // P, D), BF16, kind="ExternalInput").ap()
    out = nc.dram_tensor("out", (S, D), BF16, kind="ExternalOutput").ap()
    with tile.TileContext(nc) as tc:
        _mha_tile_kernel(tc, xT, wq, wk, wv, wo, out)
    nc.compile()
    _CACHE["nc"] = nc
    return nc


def make_in_maps(x, Wq, Wk, Wv, Wo):
    x = np.asarray(x, np.float32)
    xTb = [np.ascontiguousarray(x[b].T).astype(BFNP) for b in range(B)]
    wqs, wks, wvs, wos = [], [], [], []
    for g in range(GROUPS):
        cols = slice(g * DG, (g + 1) * DG)

        def wslice(W):
            # [D, DG] -> [128, KSUB, DG] with [p, k, m] = W[k*128+p, m]
            return np.ascontiguousarray(
                np.asarray(W, np.float32)[:, cols].reshape(KSUB, P, DG).transpose(1, 0, 2)
            ).astype(BFNP)

        wqs.append(wslice(Wq))
        wks.append(wslice(Wk))
        wvs.append(wslice(Wv))
        wos.append(
            np.ascontiguousarray(
                np.asarray(Wo, np.float32)[cols, :].reshape(DG // P, P, D).transpose(1, 0, 2)
            ).astype(BFNP)
        )
    in_maps = []
    for c in range(NCORES):
        b, g = c // GROUPS, c % GROUPS
        in_maps.append(
            {"xT": xTb[b], "wq": wqs[g], "wk": wks[g], "wv": wvs[g], "wo": wos[g]}
        )
    return in_maps


def kernel(x, Wq, Wk, Wv, Wo, bo):
    nc = build_nc()
    in_maps = make_in_maps(x, Wq, Wk, Wv, Wo)
    trace = bool(int(os.environ.get("MHA_TRACE", "0")))
    if trace:
        _install_ntff_hook()
    res = run_bass_kernel_spmd(
        nc, in_maps, core_ids=list(range(NCORES)), trace=trace,
        trace_cores=list(range(NCORES)) if trace else None,
    )
    _CACHE["last_results"] = res
    bo = np.asarray(bo, np.float32)
    out = np.zeros((B, S, D), np.float32)
    for c in range(NCORES):
        out[c // GROUPS] += res.results[c]["out"].astype(np.float32)
    out += bo[None, None, :]
    return out


# revision 6
# speedup vs baseline: 1.1935x; 1.1766x over previous
"""Multi-head causal attention (B=2, S=2048, D=1024, H=16) on 8 trn2 cores.

Sharding: core c handles batch b = c // 4 and head group g = c % 4 (4 heads,
256 feature columns). Each core computes its heads' attention context and a
partial output projection (ctx_g @ Wo[rows_g]); the host sums the 4 partials
per batch and adds bo.

v3 (all matmul operands bf16, fp32 psum accumulation):
- Demand-ordered DMAs (wq k0 first, wo last): first matmul starts ~2us in.
- Attention is sq-tile-major; projection slices and output-projection chunks
  are interleaved into the attention pair stream at matmul-chain granularity,
  so the PE always has independent dense work to hide the exp/ACT latency and
  the HAM activity monitor never downclocks.
- Within a sq-tile, ski pairs share one 2-bank psum + one wide exp ACTIVATE;
  the PV matmuls of pair i are issued after scores of pair i+1.
- Causal diag masking via DVE multiply with a precomputed triangular bf16
  mask. Evacuation copies are spread across ACT (qt), GpSimd (kt), DVE (v).
- Softmax denominator: ones column folded into V stationary; reciprocal runs
  on a DMA-scattered [128,4] layout (all DVE lanes), broadcast back across
  partitions via a DRAM round trip.
"""

import os
import sys
import types
from contextlib import ExitStack

import numpy as np
import ml_dtypes

import concourse.bacc as bacc
import concourse.bass as bass
import concourse.mybir as mybir
import concourse.tile as tile
from concourse.bass_utils import run_bass_kernel_spmd


def _install_ntff_hook():
    """The agent image's antenv lacks axon_hooks, so trn_boot's NTFF hook
    install degrades silently. Recreate the module + hook so trace=True works."""
    if "antenv.axon_hooks" in sys.modules:
        return
    try:
        mod = types.ModuleType("antenv.axon_hooks")
        holder = [None]
        mod.set_axon_ntff_profile_hook = lambda h: holder.__setitem__(0, h)
        mod.get_axon_ntff_profile_hook = lambda: holder[0]
        from trn_agent_boot.trn_boot import _ntff_profile_via_ctypes

        hook = _ntff_profile_via_ctypes("/opt/axon/libaxon_pjrt.so")
        if hook is None:
            return
        mod.set_axon_ntff_profile_hook(hook)
        sys.modules["antenv.axon_hooks"] = mod
    except Exception:
        pass

B, S, D, H, HD = 2, 2048, 1024, 16, 64
NCORES = 8
GROUPS = 4          # head groups (cores) per batch
HC = H // GROUPS    # heads per core
DG = HC * HD        # feature columns per core (256)
P = 128
KSUB = D // P       # 8 contraction subtiles for the projections
SQT = 512           # sq tile width (free dim of scores/ctx matmuls)
NSQ = S // SQT      # 4
NST = S // P        # 16 s subtiles of 128
F32 = mybir.dt.float32
BF16 = mybir.dt.bfloat16
BFNP = ml_dtypes.bfloat16

_CACHE = {}


def _mha_tile_kernel(tc, xT, wq, wk, wv, wo, out):
    nc = tc.nc
    scale = 1.0 / np.sqrt(np.float32(HD))

    with ExitStack() as ctx:
        consts = ctx.enter_context(tc.tile_pool(name="consts", bufs=1))
        dramp = ctx.enter_context(tc.tile_pool(name="dramp", bufs=3, space="DRAM"))
        # PSUM (8 banks): attn scores 2x[128,1024] + proj/outproj 2x[128,512]
        # + ctx accumulators 2x[128,512]
        psp = ctx.enter_context(tc.tile_pool(name="psp", bufs=2, space="PSUM"))
        xnp = ctx.enter_context(tc.tile_pool(name="xnp", bufs=3))
        ptp = ctx.enter_context(tc.tile_pool(name="ptp", bufs=6))
        smalls = ctx.enter_context(tc.tile_pool(name="smalls", bufs=3))
        outp = ctx.enter_context(tc.tile_pool(name="outp", bufs=3))

        # --- persistent SBUF tensors ---
        wq_sb = consts.tile([P, KSUB, DG], BF16)
        wk_sb = consts.tile([P, KSUB, DG], BF16)
        wv_sb = consts.tile([P, KSUB, DG], BF16)
        wo_sb = consts.tile([P, DG // P, D], BF16)
        qt_sb = consts.tile([P, DG // P, S], BF16)   # Q^T: head h at [64*(h%2):, h//2, :]
        # K^T zero-padded per head: head h's 64 rows at [64*(h%2):, h, :], the
        # other 64 rows 0 so score matmuls contract over K=128 (keeps the PE's
        # HAM activity monitor at full clock).
        kt_sb = consts.tile([P, HC, S], BF16)
        # V with the ones column baked in, per s-subtile and head:
        #   even h: [V(64) | 1 | 0(63)]  -> ctx rows 0-63, denom row 64
        #   odd  h: [1 | 0(63) | V(64)]  -> denom row 0, ctx rows 64-127
        v_sb = consts.tile([P, NST, HC, P], BF16)
        ctxt_sb = consts.tile([P, DG // P, S], BF16)  # normalized ctx^T, qt layout

        # demand-ordered input DMAs: wq k0 chunk, then x slice 0, then the rest
        nc.sync.dma_start(out=wq_sb[:, 0:1, :], in_=wq[:, 0:1, :])
        nc.sync.dma_start(out=wq_sb[:, 1:KSUB, :], in_=wq[:, 1:KSUB, :])

        def load_x(n):
            xn = xnp.tile([P, KSUB, SQT], BF16, tag="xT", name=f"xn_{n}")
            for k in range(KSUB):
                nc.sync.dma_start(
                    out=xn[:, k, :], in_=xT[k * P : (k + 1) * P, n * SQT : (n + 1) * SQT]
                )
            return xn

        xn0 = load_x(0)
        nc.sync.dma_start(out=wk_sb[:, 0:1, :], in_=wk[:, 0:1, :])
        nc.sync.dma_start(out=wk_sb[:, 1:KSUB, :], in_=wk[:, 1:KSUB, :])
        nc.sync.dma_start(out=wv_sb[:, 0:1, :], in_=wv[:, 0:1, :])
        nc.sync.dma_start(out=wv_sb[:, 1:KSUB, :], in_=wv[:, 1:KSUB, :])

        # --- small constants + one-time pad init (off the per-slice path) ---
        zrow = consts.tile([P, SQT], F32, tag="zrow", bufs=1)
        nc.vector.memset(zrow, 0.0)
        osc = consts.tile([P, 1], F32, tag="osc", bufs=1)
        nc.vector.memset(osc, 1.0)
        mskf = consts.tile([P, P], F32, tag="mskf", bufs=1)
        nc.vector.memset(mskf, 1.0)
        # keep entries with col >= row (sk <= sq), zero the rest
        nc.gpsimd.affine_select(
            out=mskf, in_=mskf, pattern=[[1, P]],
            compare_op=mybir.AluOpType.is_ge, fill=0.0,
            base=0, channel_multiplier=-1,
        )
        msk = consts.tile([P, P], BF16, tag="msk", bufs=1)
        nc.vector.tensor_copy(out=msk, in_=mskf)
        # kt pads: the 64 unused partitions per head stay zero forever
        nc.vector.tensor_copy(
            out=kt_sb[64:P, 0::2, :].rearrange("p h (a b) -> p h a b", b=SQT),
            in_=zrow[64:P, None, None, :].to_broadcast((64, 2, S // SQT, SQT)),
        )
        nc.vector.tensor_copy(
            out=kt_sb[0:64, 1::2, :].rearrange("p h (a b) -> p h a b", b=SQT),
            in_=zrow[0:64, None, None, :].to_broadcast((64, 2, S // SQT, SQT)),
        )
        # v pads + ones columns
        nc.vector.tensor_copy(
            out=v_sb[:, :, 0:HC:2, HD + 1 : P],
            in_=zrow[:, None, None, 0 : P - HD - 1].to_broadcast((P, NST, 2, P - HD - 1)),
        )
        nc.vector.tensor_copy(
            out=v_sb[:, :, 1:HC:2, 1:HD],
            in_=zrow[:, None, None, 0 : HD - 1].to_broadcast((P, NST, 2, HD - 1)),
        )
        for h in range(HC):
            oc = HD if h % 2 == 0 else 0
            nc.vector.tensor_copy(
                out=v_sb[:, :, h, oc : oc + 1],
                in_=osc[:, None, :].to_broadcast((P, NST, 1)),
            )

        def proj_chunks(n, xn):
            """Projection slice n as 6 independent PE chunks (Q m0, Q m1,
            K m0, K m1, V half0, V half1), each an 8..16-matmul psum chain."""
            nsl = slice(n * SQT, (n + 1) * SQT)
            for m in range(DG // P):
                ps = psp.tile([P, SQT], F32, tag="mm1b", name=f"qp_{n}_{m}")
                for k in range(KSUB):
                    nc.tensor.matmul(
                        ps, lhsT=wq_sb[:, k, m * P : (m + 1) * P], rhs=xn[:, k, :],
                        start=(k == 0), stop=(k == KSUB - 1),
                    )
                nc.scalar.copy(out=qt_sb[:, m, nsl], in_=ps)
                yield
            for m in range(DG // P):
                ps = psp.tile([P, SQT], F32, tag="mm1b", name=f"kp_{n}_{m}")
                for k in range(KSUB):
                    nc.tensor.matmul(
                        ps, lhsT=wk_sb[:, k, m * P : (m + 1) * P], rhs=xn[:, k, :],
                        start=(k == 0), stop=(k == KSUB - 1),
                    )
                nc.vector.tensor_copy(out=kt_sb[0:64, 2 * m, nsl], in_=ps[0:64, :])
                nc.vector.tensor_copy(out=kt_sb[64:P, 2 * m + 1, nsl], in_=ps[64:P, :])
                yield
            for half in range(2):
                ps = psp.tile([P, SQT], F32, tag="mm1b", name=f"vp_{n}_{half}")
                for j in range(2):
                    sst = 2 * half + j
                    for k in range(KSUB):
                        nc.tensor.matmul(
                            ps[:, j * DG : (j + 1) * DG],
                            lhsT=xn[:, k, sst * P : (sst + 1) * P],
                            rhs=wv_sb[:, k, :],
                            start=(k == 0), stop=(k == KSUB - 1),
                        )
                sta = n * (SQT // P) + 2 * half
                psv = ps.rearrange("p (t h d) -> p t h d", h=HC, d=HD)
                nc.vector.tensor_copy(
                    out=v_sb[:, sta : sta + 2, 0:HC:2, 0:HD], in_=psv[:, :, 0:HC:2, :]
                )
                nc.vector.tensor_copy(
                    out=v_sb[:, sta : sta + 2, 1:HC:2, HD:P], in_=psv[:, :, 1:HC:2, :]
                )
                yield

        def emit_norm(sqt, cp, nh):
            sq0 = sqt * SQT
            nhm, nhp = nh // 2, 64 * (nh % 2)
            ncr = 0 if nh % 2 == 0 else 64
            ndr = 64 if nh % 2 == 0 else 0
            # scatter the psum denom row across partitions so reciprocal uses
            # all DVE lanes, then broadcast 1/den back via a DRAM round trip
            rt = smalls.tile([1, SQT], F32, tag="rt", name=f"rt_{sqt}_{nh}")
            nc.scalar.copy(out=rt, in_=cp[ndr : ndr + 1, :])
            spread = smalls.tile([P, SQT // P], F32, tag="spread", name=f"sp_{sqt}_{nh}")
            nc.sync.dma_start(out=spread, in_=rt)
            nc.vector.reciprocal(out=spread, in_=spread)
            rec_d = dramp.tile([1, SQT], F32, tag="rec", name=f"rec_{sqt}_{nh}")
            nc.sync.dma_start(
                out=rec_d.rearrange("a (p f) -> (a p) f", p=P), in_=spread
            )
            bcast = smalls.tile([P, SQT], F32, tag="bcast", name=f"bc_{sqt}_{nh}")
            rec_b = bass.AP(
                tensor=rec_d.tensor, offset=rec_d.offset,
                ap=[[0, 64]] + [list(p) for p in rec_d.ap[1:]],
            )
            nc.sync.dma_start(out=bcast[ncr : ncr + 64, :], in_=rec_b)
            nc.vector.tensor_tensor(
                ctxt_sb[nhp : nhp + 64, nhm, sq0 : sq0 + SQT],
                cp[ncr : ncr + 64, :],
                bcast[ncr : ncr + 64, :],
                mybir.AluOpType.mult,
            )

        def attention_pairs(sqt):
            """sq-tile sqt, all 4 heads; yields at pair boundaries. PV of pair
            i is emitted after scores+exp of pair i+1 (also across heads)."""
            sq0 = sqt * SQT
            nsk = 4 * sqt + 4
            pending = None   # (infos, pt, cpsum, head_last) awaiting PV
            norm_q = []      # cpsum awaiting norm emission

            def emit_pv(p):
                infos, pt, cpsum, h_own = p
                for ski, w0, base in infos:
                    nc.tensor.matmul(
                        cpsum[:, w0:],
                        lhsT=v_sb[:, ski, h_own, :],
                        rhs=pt[:, base + w0 : base + SQT],
                        start=(ski == 0), stop=(ski == nsk - 1),
                    )

            for h in range(HC):
                hm = h // 2
                cpsum = psp.tile([P, SQT], F32, tag="ctx", name=f"ctx_{sqt}_{h}")
                for sk0 in range(0, nsk, 2):
                    spsum = psp.tile([P, 2 * SQT], F32, tag="s", name=f"s_{sqt}_{h}_{sk0}")
                    pt = ptp.tile([P, 2 * SQT], BF16, tag="pt", name=f"pt_{sqt}_{h}_{sk0}")
                    infos = []
                    for jj in range(2):
                        ski = sk0 + jj
                        diag = ski >= 4 * sqt
                        w0 = (128 * ski - sq0) if diag else 0
                        base = jj * SQT
                        nc.tensor.matmul(
                            spsum[:, base + w0 : base + SQT],
                            lhsT=kt_sb[:, h, ski * P : (ski + 1) * P],
                            rhs=qt_sb[:, hm, sq0 + w0 : sq0 + SQT],
                            start=True, stop=True,
                        )
                        infos.append((ski, w0, base, diag))
                    w0g = infos[0][1]
                    nc.scalar.activation(
                        out=pt[:, w0g : 2 * SQT], in_=spsum[:, w0g : 2 * SQT],
                        func=mybir.ActivationFunctionType.Exp,
                        bias=0.0, scale=float(scale),
                    )
                    for ski, w0, base, diag in infos:
                        if diag:
                            nc.vector.tensor_tensor(
                                pt[:, base + w0 : base + w0 + P],
                                pt[:, base + w0 : base + w0 + P],
                                msk, mybir.AluOpType.mult,
                            )
                    if pending is not None:
                        emit_pv(pending)
                        if pending[0][-1][0] == nsk - 1:  # finished a head
                            norm_q.append((pending[2], pending[3]))
                    while norm_q:
                        cp, nh = norm_q.pop(0)
                        emit_norm(sqt, cp, nh)
                    pending = ([(ski, w0, base) for ski, w0, base, _ in infos], pt, cpsum, h)
                    yield
            if pending is not None:
                emit_pv(pending)
                emit_norm(sqt, pending[2], pending[3])
                pending = None

        def outproj_chunks(sqt):
            """Output projection for s rows [sqt*512, (sqt+1)*512): 8 chunks."""
            for st in range(4 * sqt, 4 * sqt + 4):
                ot = outp.tile([P, D], BF16, tag="out", name=f"ot_{st}")
                for nn in range(D // SQT):
                    ps = psp.tile([P, SQT], F32, tag="mm1b", name=f"op_{st}_{nn}")
                    for k in range(DG // P):
                        nc.tensor.matmul(
                            ps,
                            lhsT=ctxt_sb[:, k, st * P : (st + 1) * P],
                            rhs=wo_sb[:, k, nn * SQT : (nn + 1) * SQT],
                            start=(k == 0), stop=(k == DG // P - 1),
                        )
                    nc.vector.tensor_copy(out=ot[:, nn * SQT : (nn + 1) * SQT], in_=ps)
                    if nn == D // SQT - 1:
                        nc.scalar.dma_start(out=out[st * P : (st + 1) * P, :], in_=ot)
                    yield

        # --- main schedule ---
        import itertools

        def drive(att, fills):
            """att: (generator, n_yields); fills: list of (generator, n_yields).
            Spreads fill chunks evenly across attention pairs."""
            agen, na = att
            fgen = itertools.chain(*[g for g, _ in fills])
            nf = sum(n for _, n in fills)
            acc = 0.0
            step = nf / na if na else 0
            done_f = 0
            for _ in agen:
                acc += step
                while done_f < int(acc + 1e-9):
                    if next(fgen, None) is None:
                        break
                    done_f += 1
            for _ in fgen:
                pass

        # fill the pipe: slice 0 projections run dense
        for _ in proj_chunks(0, xn0):
            pass
        xn1 = load_x(1)
        xn2 = load_x(2)
        drive((attention_pairs(0), 8), [(proj_chunks(1, xn1), 6)])
        nc.sync.dma_start(out=wo_sb, in_=wo)
        xn3 = load_x(3)
        drive((attention_pairs(1), 16), [(proj_chunks(2, xn2), 6), (outproj_chunks(0), 8)])
        drive((attention_pairs(2), 24), [(proj_chunks(3, xn3), 6), (outproj_chunks(1), 8)])
        drive((attention_pairs(3), 32), [(outproj_chunks(2), 8)])
        for _ in outproj_chunks(3):
            pass


def build_nc():
    if "nc" in _CACHE:
        return _CACHE["nc"]
    nc = bacc.Bacc("TRN2", target_bir_lowering=False, debug=False, num_devices=NCORES)
    xT = nc.dram_tensor("xT", (D, S), BF16, kind="ExternalInput").ap()
    wq = nc.dram_tensor("wq", (P, KSUB, DG), BF16, kind="ExternalInput").ap()
    wk = nc.dram_tensor("wk", (P, KSUB, DG), BF16, kind="ExternalInput").ap()
    wv = nc.dram_tensor("wv", (P, KSUB, DG), BF16, kind="ExternalInput").ap()
    wo = nc.dram_tensor("wo", (P, DG // P, D), BF16, kind="ExternalInput").ap()
    out = nc.dram_tensor("out", (S, D), BF16, kind="ExternalOutput").ap()
    with tile.TileContext(nc) as tc:
        _mha_tile_kernel(tc, xT, wq, wk, wv, wo, out)
    nc.compile()
    _CACHE["nc"] = nc
    return nc


def make_in_maps(x, Wq, Wk, Wv, Wo):
    x = np.asarray(x, np.float32)
    xTb = [np.ascontiguousarray(x[b].T).astype(BFNP) for b in range(B)]
    wqs, wks, wvs, wos = [], [], [], []
    for g in range(GROUPS):
        cols = slice(g * DG, (g + 1) * DG)

        def wslice(W):
            # [D, DG] -> [128, KSUB, DG] with [p, k, m] = W[k*128+p, m]
            return np.ascontiguousarray(
                np.asarray(W, np.float32)[:, cols].reshape(KSUB, P, DG).transpose(1, 0, 2)
            ).astype(BFNP)

        wqs.append(wslice(Wq))
        wks.append(wslice(Wk))
        wvs.append(wslice(Wv))
        wos.append(
            np.ascontiguousarray(
                np.asarray(Wo, np.float32)[cols, :].reshape(DG // P, P, D).transpose(1, 0, 2)
            ).astype(BFNP)
        )
    in_maps = []
    for c in range(NCORES):
        b, g = c // GROUPS, c % GROUPS
        in_maps.append(
            {"xT": xTb[b], "wq": wqs[g], "wk": wks[g], "wv": wvs[g], "wo": wos[g]}
        )
    return in_maps


def kernel(x, Wq, Wk, Wv, Wo, bo):
    nc = build_nc()
    in_maps = make_in_maps(x, Wq, Wk, Wv, Wo)
    trace = bool(int(os.environ.get("MHA_TRACE", "0")))
    if trace:
        _install_ntff_hook()
    res = run_bass_kernel_spmd(
        nc, in_maps, core_ids=list(range(NCORES)), trace=trace,
        trace_cores=list(range(NCORES)) if trace else None,
    )
    _CACHE["last_results"] = res
    bo = np.asarray(bo, np.float32)
    out = np.zeros((B, S, D), np.float32)
    for c in range(NCORES):
        out[c // GROUPS] += res.results[c]["out"].astype(np.float32)
    out += bo[None, None, :]
    return out
